# revision 13
# baseline (speedup 1.0000x reference)
"""MoE top-2 routed SwiGLU MLP on 8 Trainium2 NeuronCores.

Strategy (expert parallelism):
  - 8 experts, 8 cores: core e owns expert e's weights.
  - Host-side dispatch: gather the (unique) tokens routed to each expert,
    pack feature-major (C = max token count over experts, zero padded),
    cast to bf16.  The top-2 combine weight is folded into the up-proj
    input copy (the u-path is linear in x), so the device output is
    already combine-weighted.
  - Device (per core): dense SwiGLU MLP, everything feature-on-partition,
    tokens on the moving/free dim; all matmuls bf16 with fp32 PSUM accum:
        g = Wg^T x          accumulate over 8 H-tiles of 128
        u = Wu^T (x*comb)
        h = silu(g) * u     [2816, C] bf16 in SBUF
        y = Wd^T h          [1024, C] f32 -> DRAM
  - Packed input layouts so DMA transfer order == PE consumption order
    with few large transfers (the DMA fabric is one serial ~360GB/s pipe):
      wg/wu: [128, 22528]  col (ic*1024 + h*128 + c) = W[ic*128+c, h*128+p]
      xg/xu: [128, 8*C]    col (h*C + t) = x[t, h*128+p]
  - Host-side combine: out[tokens_e] += y_e^T (token lists are unique per
    expert; experts summed sequentially).
"""

import os
import sys

for _p in ("/opt/trn_rl_repo",):
    if _p not in sys.path and os.path.isdir(_p):
        sys.path.insert(0, _p)

from contextlib import ExitStack

import ml_dtypes
import numpy as np

import concourse.bass as bass  # noqa: F401  (engine API comes via nc)
import concourse.tile as tile
from concourse import bacc, mybir
from concourse.bass_utils import run_bass_kernel_spmd

# Problem shape (hardcoded per task instructions).
B, S, H, I, E, TOPK = 1, 2048, 1024, 2816, 8, 2
N_CORES = 8
HT = H // 128   # 8 h-tiles
IT = I // 128   # 22 i-tiles
IC_COLS = HT * 128  # packed weight cols per i-tile block

_BF16 = ml_dtypes.bfloat16

# Compiled-program cache keyed by (C, chunks, n_iter) so repeated kernel()
# calls with the same routing shape skip rebuild/recompile.
_PROG_CACHE: dict = {}


def _chunk_sizes(C: int) -> tuple[int, ...]:
    """Split C token columns into chunks of <=512 (PSUM fp32 bank limit),
    balanced and 8-aligned (C itself must be 8-aligned)."""
    nch = -(-C // 512)
    per = -(-C // nch // 8) * 8
    sizes = []
    left = C
    for _ in range(nch):
        s = min(per, left)
        sizes.append(s)
        left -= s
    assert left == 0 and all(s > 0 for s in sizes)
    return tuple(sizes)


def _build_program(C: int, chunks: tuple[int, ...], n_iter: int = 1,
                   ic_bounds: tuple[int, ...] = tuple(range(1, IT + 1))):
    """Build + compile the per-core SPMD Bass program.

    n_iter > 1 wraps the body in a Tile For_i loop; used only for
    differential hardware timing (the output is unchanged since every
    iteration recomputes the same thing)."""
    nc = bacc.Bacc(
        "TRN2",
        target_bir_lowering=False,
        debug=False,
        enable_asserts=False,
        num_devices=N_CORES,
    )
    bf16 = mybir.dt.bfloat16
    f32 = mybir.dt.float32
    XW = HT * C
    WW = HT * I

    wg_d = nc.dram_tensor("wg", [128, WW], bf16, kind="ExternalInput").ap()
    wu_d = nc.dram_tensor("wu", [128, WW], bf16, kind="ExternalInput").ap()
    wd_d = nc.dram_tensor("wd", [I, H], bf16, kind="ExternalInput").ap()
    xg_d = nc.dram_tensor("xg", [128, XW], bf16, kind="ExternalInput").ap()
    xu_d = nc.dram_tensor("xu", [128, XW], bf16, kind="ExternalInput").ap()
    y_d = nc.dram_tensor("y", [H, C], f32, kind="ExternalOutput").ap()

    offs = []
    o = 0
    for n in chunks:
        offs.append((o, n))
        o += n

    with ExitStack() as ctx:
        tc = ctx.enter_context(tile.TileContext(nc))
        wpool = ctx.enter_context(tc.tile_pool(name="w", bufs=1))
        xpool = ctx.enter_context(tc.tile_pool(name="x", bufs=1))
        hpool = ctx.enter_context(tc.tile_pool(name="hbuf", bufs=1))
        spool = ctx.enter_context(tc.tile_pool(name="s", bufs=2))
        ypool = ctx.enter_context(tc.tile_pool(name="yst", bufs=2))
        psum = ctx.enter_context(tc.tile_pool(name="ps", bufs=2, space="PSUM"))

        xg_sb = xpool.tile([128, XW], bf16, name="xga")
        xu_sb = xpool.tile([128, XW], bf16, name="xua")
        wg_sb = wpool.tile([128, WW], bf16, name="wga")
        wu_sb = wpool.tile([128, WW], bf16, name="wua")
        wd_sb = [wpool.tile([128, H], bf16, tag=f"wd{i}", name=f"wd{i}") for i in range(IT)]
        h_sb = [hpool.tile([128, C], bf16, tag=f"h{i}", name=f"hb{i}") for i in range(IT)]

        if n_iter > 1:
            ctx.enter_context(tc.For_i(0, n_iter, 1))

        # One HWDGE queue, transfers emitted in exact consumption order.
        half = (HT // 2) * C
        nc.sync.dma_start(wg_sb[:, 0:IC_COLS], wg_d[:, 0:IC_COLS])
        nc.sync.dma_start(xg_sb[:, 0:half], xg_d[:, 0:half])
        nc.sync.dma_start(xg_sb[:, half:XW], xg_d[:, half:XW])
        nc.sync.dma_start(wu_sb[:, 0:IC_COLS], wu_d[:, 0:IC_COLS])
        nc.sync.dma_start(xu_sb[:, 0:half], xu_d[:, 0:half])
        nc.sync.dma_start(xu_sb[:, half:XW], xu_d[:, half:XW])
        ic_bounds = list(ic_bounds)
        assert ic_bounds[-1] == IT
        for g in range(len(ic_bounds) - 1):
            cols = slice(ic_bounds[g] * IC_COLS, ic_bounds[g + 1] * IC_COLS)
            nc.sync.dma_start(wg_sb[:, cols], wg_d[:, cols])
            nc.sync.dma_start(wu_sb[:, cols], wu_d[:, cols])
        for i in range(IT):
            nc.sync.dma_start(wd_sb[i][:], wd_d[slice(i * 128, (i + 1) * 128), :])

        # Phase 1: gate/up projections + silu*mul, one i-tile at a time.
        for ic in range(IT):
            pg = [psum.tile([128, n], f32, tag=f"pg{c}", name=f"pg{c}") for c, (_, n) in enumerate(offs)]
            pu = [psum.tile([128, n], f32, tag=f"pu{c}", name=f"pu{c}") for c, (_, n) in enumerate(offs)]
            for h in range(HT):
                wcol = ic * IC_COLS + h * 128
                lwg = wg_sb[:, wcol:wcol + 128]
                for c, (o_, n) in enumerate(offs):
                    nc.tensor.matmul(
                        pg[c][:], lwg, xg_sb[:, h * C + o_ : h * C + o_ + n],
                        start=(h == 0), stop=(h == HT - 1),
                    )
            for h in range(HT):
                wcol = ic * IC_COLS + h * 128
                lwu = wu_sb[:, wcol:wcol + 128]
                for c, (o_, n) in enumerate(offs):
                    nc.tensor.matmul(
                        pu[c][:], lwu, xu_sb[:, h * C + o_ : h * C + o_ + n],
                        start=(h == 0), stop=(h == HT - 1),
                    )
            for c, (o_, n) in enumerate(offs):
                sg = spool.tile([128, n], f32, tag=f"sg{c}", name=f"sg{c}")
                nc.scalar.activation(
                    sg[:], pg[c][:], mybir.ActivationFunctionType.Silu
                )
                nc.vector.tensor_mul(h_sb[ic][:, o_ : o_ + n], sg[:], pu[c][:])

        # Phase 2: down projection, one output h-tile at a time.
        for hc in range(HT):
            hcc = slice(hc * 128, (hc + 1) * 128)
            py = [psum.tile([128, n], f32, tag=f"pg{c}", name=f"pg{c}") for c, (_, n) in enumerate(offs)]
            for i in range(IT):
                lw = wd_sb[i][:, hcc]
                for c, (o_, n) in enumerate(offs):
                    nc.tensor.matmul(
                        py[c][:], lw, h_sb[i][:, o_ : o_ + n],
                        start=(i == 0), stop=(i == IT - 1),
                    )
            y_sb = ypool.tile([128, C], f32, tag="y", name="ysb")
            for c, (o_, n) in enumerate(offs):
                nc.vector.tensor_copy(y_sb[:, o_ : o_ + n], py[c][:])
                nc.sync.dma_start(y_d[hcc, o_ : o_ + n], y_sb[:, o_ : o_ + n])

    nc.compile()
    return nc


def _pack_w(w_t: np.ndarray) -> np.ndarray:
    """[I, H] expert weight -> packed [128, IT*HT*128] bf16 with
    col (ic*1024 + h*128 + c) at partition p = W[ic*128+c, h*128+p]."""
    return np.ascontiguousarray(
        w_t.reshape(IT, 128, HT, 128).transpose(3, 0, 2, 1).reshape(128, IT * HT * 128)
    ).astype(_BF16)


def _pack_x(xe: np.ndarray, C: int) -> np.ndarray:
    """[n, H] token rows -> packed [128, HT*C] bf16 with col (h*C + t) at
    partition p = x[t, h*128+p]."""
    n = xe.shape[0]
    out = np.zeros((128, HT * C), _BF16)
    # [n, HT, 128] -> [128, HT, n]
    blk = xe.reshape(n, HT, 128).transpose(2, 1, 0).astype(_BF16)
    out.reshape(128, HT, C)[:, :, :n] = blk
    return out


def _prepare(x, expert_indices, expert_weights, gate_proj, up_proj, down_proj):
    """Host-side dispatch.  Returns (C, chunks, in_maps, token_lists)."""
    x_flat = np.asarray(x, dtype=np.float32).reshape(-1, H)
    T = x_flat.shape[0]
    idx = np.asarray(expert_indices).reshape(T, TOPK).astype(np.int64)
    w = np.asarray(expert_weights, dtype=np.float32).reshape(T, TOPK)

    comb = np.zeros((T, E), np.float32)
    np.add.at(comb, (np.arange(T)[:, None], idx), w)
    assigned = np.zeros((T, E), bool)
    assigned[np.arange(T)[:, None], idx] = True

    token_lists = [np.nonzero(assigned[:, e])[0] for e in range(E)]
    cmax = max(len(t) for t in token_lists)
    C = max(-(-cmax // 8) * 8, 64)
    chunks = _chunk_sizes(C)

    gate = np.asarray(gate_proj, dtype=np.float32)
    up = np.asarray(up_proj, dtype=np.float32)
    down = np.asarray(down_proj, dtype=np.float32)

    in_maps = []
    for e in range(E):
        tok = token_lists[e]
        xe = x_flat[tok]                          # [n, H] f32
        in_maps.append(
            {
                "wg": _pack_w(gate[e]),
                "wu": _pack_w(up[e]),
                "wd": np.ascontiguousarray(down[e].T).astype(_BF16),  # [I, H]
                "xg": _pack_x(xe, C),
                "xu": _pack_x(xe * comb[tok, e][:, None], C),
            }
        )
    return C, chunks, in_maps, token_lists


def kernel(x, expert_indices, expert_weights, gate_proj, up_proj, down_proj):
    C, chunks, in_maps, token_lists = _prepare(
        x, expert_indices, expert_weights, gate_proj, up_proj, down_proj
    )
    key = (C, chunks, 1)
    if key not in _PROG_CACHE:
        _PROG_CACHE[key] = _build_program(C, chunks)
    nc = _PROG_CACHE[key]

    res = run_bass_kernel_spmd(nc, in_maps, core_ids=list(range(N_CORES)))

    T = B * S
    out_flat = np.zeros((T, H), np.float32)
    for e in range(E):
        tok = token_lists[e]
        y = res.results[e]["y"]                   # [H, C] f32
        out_flat[tok] += y[:, : len(tok)].T
    return out_flat.reshape(B, S, H)


# revision 18
# speedup vs baseline: 1.0562x; 1.0562x over previous
"""MoE top-2 routed SwiGLU MLP on 8 Trainium2 NeuronCores.

Strategy (expert parallelism):
  - 8 experts, 8 cores: core e owns expert e's weights.
  - Host-side dispatch: gather the (unique) tokens routed to each expert,
    pack feature-major (C = max token count over experts, zero padded),
    cast to bf16.  The top-2 combine weight is folded into the up-proj
    input copy (the u-path is linear in x), so the device output is
    already combine-weighted.
  - Device (per core): dense SwiGLU MLP, everything feature-on-partition,
    tokens on the moving/free dim; all matmuls bf16 with fp32 PSUM accum:
        g = Wg^T x          accumulate over 8 H-tiles of 128
        u = Wu^T (x*comb)
        h = silu(g) * u     [2816, C] bf16 in SBUF
        y = Wd^T h          [1024, C] f32 -> DRAM
  - Packed input layouts so DMA transfer order == PE consumption order
    with few large transfers (the DMA fabric is one serial ~360GB/s pipe):
      wg/wu: [128, 22528]  col (ic*1024 + h*128 + c) = W[ic*128+c, h*128+p]
      xg/xu: [128, 8*C]    col (h*C + t) = x[t, h*128+p]
  - Host-side combine: out[tokens_e] += y_e^T (token lists are unique per
    expert; experts summed sequentially).
"""

import os
import sys

for _p in ("/opt/trn_rl_repo",):
    if _p not in sys.path and os.path.isdir(_p):
        sys.path.insert(0, _p)

from contextlib import ExitStack

import ml_dtypes
import numpy as np

import concourse.bass as bass  # noqa: F401  (engine API comes via nc)
import concourse.tile as tile
from concourse import bacc, mybir
from concourse.bass_utils import run_bass_kernel_spmd

# Problem shape (hardcoded per task instructions).
B, S, H, I, E, TOPK = 1, 2048, 1024, 2816, 8, 2
N_CORES = 8
HT = H // 128   # 8 h-tiles
IT = I // 128   # 22 i-tiles
IC_COLS = HT * 128  # packed weight cols per i-tile block

_BF16 = ml_dtypes.bfloat16

# Compiled-program cache keyed by (C, chunks, n_iter) so repeated kernel()
# calls with the same routing shape skip rebuild/recompile.
_PROG_CACHE: dict = {}


def _chunk_sizes(C: int) -> tuple[int, ...]:
    """Split C token columns into chunks of <=512 (PSUM fp32 bank limit),
    balanced and 8-aligned (C itself must be 8-aligned)."""
    nch = -(-C // 512)
    per = -(-C // nch // 8) * 8
    sizes = []
    left = C
    for _ in range(nch):
        s = min(per, left)
        sizes.append(s)
        left -= s
    assert left == 0 and all(s > 0 for s in sizes)
    return tuple(sizes)


def _build_program(C: int, chunks: tuple[int, ...], n_iter: int = 1,
                   ic_bounds: tuple[int, ...] = tuple(range(1, IT + 1)),
                   style: str = "default"):
    """Build + compile the per-core SPMD Bass program.

    n_iter > 1 wraps the body in a Tile For_i loop; used only for
    differential hardware timing (the output is unchanged since every
    iteration recomputes the same thing)."""
    nc = bacc.Bacc(
        "TRN2",
        target_bir_lowering=False,
        debug=False,
        enable_asserts=False,
        num_devices=N_CORES,
    )
    bf16 = mybir.dt.bfloat16
    f32 = mybir.dt.float32
    XW = HT * C
    WW = HT * I

    wg_d = nc.dram_tensor("wg", [128, WW], bf16, kind="ExternalInput").ap()
    wu_d = nc.dram_tensor("wu", [128, WW], bf16, kind="ExternalInput").ap()
    wd_d = nc.dram_tensor("wd", [I, H], bf16, kind="ExternalInput").ap()
    xg_d = nc.dram_tensor("xg", [128, XW], bf16, kind="ExternalInput").ap()
    xu_d = nc.dram_tensor("xu", [128, XW], bf16, kind="ExternalInput").ap()
    y_d = nc.dram_tensor("y", [H, C], f32, kind="ExternalOutput").ap()

    offs = []
    o = 0
    for n in chunks:
        offs.append((o, n))
        o += n
    # Single-chunk programs only need 2 live PSUM tags -> deepen buffering.
    psum_bufs = 4 if len(chunks) == 1 else 2

    with ExitStack() as ctx:
        tc = ctx.enter_context(tile.TileContext(nc))
        wpool = ctx.enter_context(tc.tile_pool(name="w", bufs=1))
        xpool = ctx.enter_context(tc.tile_pool(name="x", bufs=1))
        hpool = ctx.enter_context(tc.tile_pool(name="hbuf", bufs=1))
        spool = ctx.enter_context(tc.tile_pool(name="s", bufs=2))
        ypool = ctx.enter_context(tc.tile_pool(name="yst", bufs=2))
        psum = ctx.enter_context(tc.tile_pool(name="ps", bufs=2, space="PSUM"))

        xg_sb = xpool.tile([128, XW], bf16, name="xga")
        xu_sb = xpool.tile([128, XW], bf16, name="xua")
        wg_sb = wpool.tile([128, WW], bf16, name="wga")
        wu_sb = wpool.tile([128, WW], bf16, name="wua")
        wd_sb = [wpool.tile([128, H], bf16, tag=f"wd{i}", name=f"wd{i}") for i in range(IT)]
        h_sb = [hpool.tile([128, C], bf16, tag=f"h{i}", name=f"hb{i}") for i in range(IT)]

        if n_iter > 1:
            ctx.enter_context(tc.For_i(0, n_iter, 1))

        # One HWDGE queue, transfers emitted in exact consumption order.
        half = (HT // 2) * C
        nc.sync.dma_start(wg_sb[:, 0:IC_COLS], wg_d[:, 0:IC_COLS])
        nc.sync.dma_start(xg_sb[:, 0:half], xg_d[:, 0:half])
        nc.sync.dma_start(xg_sb[:, half:XW], xg_d[:, half:XW])
        nc.sync.dma_start(wu_sb[:, 0:IC_COLS], wu_d[:, 0:IC_COLS])
        nc.sync.dma_start(xu_sb[:, 0:half], xu_d[:, 0:half])
        nc.sync.dma_start(xu_sb[:, half:XW], xu_d[:, half:XW])
        ic_bounds = list(ic_bounds)
        assert ic_bounds[-1] == IT
        for g in range(len(ic_bounds) - 1):
            cols = slice(ic_bounds[g] * IC_COLS, ic_bounds[g + 1] * IC_COLS)
            nc.sync.dma_start(wg_sb[:, cols], wg_d[:, cols])
            nc.sync.dma_start(wu_sb[:, cols], wu_d[:, cols])
        for i in range(IT):
            nc.sync.dma_start(wd_sb[i][:], wd_d[slice(i * 128, (i + 1) * 128), :])

        # Phase 1: gate/up projections + silu*mul, one i-tile at a time.
        for ic in range(IT):
            pg = [psum.tile([128, n], f32, tag=f"pg{c}", name=f"pg{c}", bufs=psum_bufs) for c, (_, n) in enumerate(offs)]
            pu = [psum.tile([128, n], f32, tag=f"pu{c}", name=f"pu{c}", bufs=psum_bufs) for c, (_, n) in enumerate(offs)]
            for h in range(HT):
                wcol = ic * IC_COLS + h * 128
                lwg = wg_sb[:, wcol:wcol + 128]
                for c, (o_, n) in enumerate(offs):
                    nc.tensor.matmul(
                        pg[c][:], lwg, xg_sb[:, h * C + o_ : h * C + o_ + n],
                        start=(h == 0), stop=(h == HT - 1),
                    )
            for h in range(HT):
                wcol = ic * IC_COLS + h * 128
                lwu = wu_sb[:, wcol:wcol + 128]
                for c, (o_, n) in enumerate(offs):
                    nc.tensor.matmul(
                        pu[c][:], lwu, xu_sb[:, h * C + o_ : h * C + o_ + n],
                        start=(h == 0), stop=(h == HT - 1),
                    )
            for c, (o_, n) in enumerate(offs):
                if style == "mmonly":
                    nc.vector.tensor_copy(h_sb[ic][:, o_ : o_ + n], pu[c][:])
                else:
                    sg = spool.tile([128, n], f32, tag=f"sg{c}", name=f"sg{c}")
                    nc.scalar.activation(
                        sg[:], pg[c][:], mybir.ActivationFunctionType.Silu
                    )
                    nc.vector.tensor_mul(h_sb[ic][:, o_ : o_ + n], sg[:], pu[c][:])

        # Phase 2: down projection, one output h-tile at a time.
        for hc in range(HT):
            hcc = slice(hc * 128, (hc + 1) * 128)
            py = [psum.tile([128, n], f32, tag=f"pg{c}", name=f"pg{c}", bufs=psum_bufs) for c, (_, n) in enumerate(offs)]
            for i in range(IT):
                lw = wd_sb[i][:, hcc]
                for c, (o_, n) in enumerate(offs):
                    nc.tensor.matmul(
                        py[c][:], lw, h_sb[i][:, o_ : o_ + n],
                        start=(i == 0), stop=(i == IT - 1),
                    )
            y_sb = ypool.tile([128, C], f32, tag="y", name="ysb")
            for c, (o_, n) in enumerate(offs):
                nc.vector.tensor_copy(y_sb[:, o_ : o_ + n], py[c][:])
                nc.sync.dma_start(y_d[hcc, o_ : o_ + n], y_sb[:, o_ : o_ + n])

    nc.compile()
    return nc


def _pack_w(w_t: np.ndarray) -> np.ndarray:
    """[I, H] expert weight -> packed [128, IT*HT*128] bf16 with
    col (ic*1024 + h*128 + c) at partition p = W[ic*128+c, h*128+p]."""
    return np.ascontiguousarray(
        w_t.reshape(IT, 128, HT, 128).transpose(3, 0, 2, 1).reshape(128, IT * HT * 128)
    ).astype(_BF16)


def _pack_x(xe: np.ndarray, C: int) -> np.ndarray:
    """[n, H] token rows -> packed [128, HT*C] bf16 with col (h*C + t) at
    partition p = x[t, h*128+p]."""
    n = xe.shape[0]
    out = np.zeros((128, HT * C), _BF16)
    # [n, HT, 128] -> [128, HT, n]
    blk = xe.reshape(n, HT, 128).transpose(2, 1, 0).astype(_BF16)
    out.reshape(128, HT, C)[:, :, :n] = blk
    return out


def _prepare(x, expert_indices, expert_weights, gate_proj, up_proj, down_proj):
    """Host-side dispatch.  Returns (C, chunks, in_maps, token_lists)."""
    x_flat = np.asarray(x, dtype=np.float32).reshape(-1, H)
    T = x_flat.shape[0]
    idx = np.asarray(expert_indices).reshape(T, TOPK).astype(np.int64)
    w = np.asarray(expert_weights, dtype=np.float32).reshape(T, TOPK)

    comb = np.zeros((T, E), np.float32)
    np.add.at(comb, (np.arange(T)[:, None], idx), w)
    assigned = np.zeros((T, E), bool)
    assigned[np.arange(T)[:, None], idx] = True

    token_lists = [np.nonzero(assigned[:, e])[0] for e in range(E)]
    cmax = max(len(t) for t in token_lists)
    C = max(-(-cmax // 8) * 8, 64)
    # A single 512-token chunk halves the matmul count vs two chunks (the
    # per-matmul fixed overhead is what keeps us off the PE roofline).  If
    # only a few tokens spill past 512, compute those on the host in exact
    # fp32 (<=0.5% of the FLOPs) and cap the device batch at 512.
    overflow_lists = [np.empty(0, np.int64) for _ in range(E)]
    if C > 512 and sum(max(0, len(t) - 512) for t in token_lists) <= 64:
        overflow_lists = [t[512:] for t in token_lists]
        token_lists = [t[:512] for t in token_lists]
        C = 512
    chunks = _chunk_sizes(C)

    gate = np.asarray(gate_proj, dtype=np.float32)
    up = np.asarray(up_proj, dtype=np.float32)
    down = np.asarray(down_proj, dtype=np.float32)

    in_maps = []
    for e in range(E):
        tok = token_lists[e]
        xe = x_flat[tok]                          # [n, H] f32
        in_maps.append(
            {
                "wg": _pack_w(gate[e]),
                "wu": _pack_w(up[e]),
                "wd": np.ascontiguousarray(down[e].T).astype(_BF16),  # [I, H]
                "xg": _pack_x(xe, C),
                "xu": _pack_x(xe * comb[tok, e][:, None], C),
            }
        )
    return C, chunks, in_maps, token_lists, overflow_lists, comb


def _sigmoid(v):
    return 1.0 / (1.0 + np.exp(-v))


def kernel(x, expert_indices, expert_weights, gate_proj, up_proj, down_proj):
    C, chunks, in_maps, token_lists, overflow_lists, comb = _prepare(
        x, expert_indices, expert_weights, gate_proj, up_proj, down_proj
    )
    key = (C, chunks, 1)
    if key not in _PROG_CACHE:
        _PROG_CACHE[key] = _build_program(C, chunks)
    nc = _PROG_CACHE[key]

    res = run_bass_kernel_spmd(nc, in_maps, core_ids=list(range(N_CORES)))

    T = B * S
    x_flat = np.asarray(x, dtype=np.float32).reshape(T, H)
    out_flat = np.zeros((T, H), np.float32)
    for e in range(E):
        tok = token_lists[e]
        y = res.results[e]["y"]                   # [H, C] f32
        out_flat[tok] += y[:, : len(tok)].T
        ovf = overflow_lists[e]
        if len(ovf):
            ge = np.asarray(gate_proj, dtype=np.float32)[e]
            ue = np.asarray(up_proj, dtype=np.float32)[e]
            de = np.asarray(down_proj, dtype=np.float32)[e]
            xo = x_flat[ovf]
            g = xo @ ge.T
            u = xo @ ue.T
            h = (g * _sigmoid(g)) * u
            out_flat[ovf] += (comb[ovf, e][:, None] * (h @ de.T))
    return out_flat.reshape(B, S, H)
